# revision 15
# baseline (speedup 1.0000x reference)
"""Trainium2 Bass kernel for nn_DecoderTransformerBackbone_1589137900084.

Decoder transformer backbone: B=8, N=2048, D=256, L=4 layers of
relu-attention with a causal averaging mask + MLP, layernorms after each
residual. Data-parallel over batch: one batch element per NeuronCore (8 cores).

Per-core layout strategy:
  - H (the residual stream) lives in SBUF in normal layout as 16 tiles of
    [128 tokens, 256 dims].
  - Each layer PE-transposes H into HT [d, n] twice (for QKV and for the MLP),
    batching 4 [128,128] transposes per PSUM bank to amortize copy overhead.
  - qT/kT are produced transposed ([e, n]) with the weights stationary; v and
    the MLP2 output are produced in normal layout with activation tiles
    stationary.
  - S^T tiles ([j, i], 128x512) are computed only for the lower-triangular
    blocks; relu is fused into the PSUM->SBUF copy; the in-block triangle is
    applied by one [128,128] mask multiply on diagonal blocks; the 1/(i+1) row
    scale is fused into the attention residual via scalar_tensor_tensor.
  - All big matmuls run as float32r (single-pass fp32, ~1e-4 relative error);
    PE transposes run exact fp32.
"""
import sys

sys.path.insert(0, "/opt/trn_rl_repo")

import numpy as np

B, N, D, L = 8, 2048, 256, 4
LN_EPS = 1e-5
P = 128
NT = N // P            # 16 token tiles
DT = D // P            # 2 dim tiles
IC = N // 512          # 4 free-dim chunks of 512

_CACHE = {}
_last_in_maps = None
TRI_ON_GPSIMD = False
REPEAT = 1
PROFILE = False
LAST_EXEC_NS = None
LAST_RESULTS = None


def _build_program(use_b1, use_b2, use_ln1_gb, use_ln2_gb):
    import concourse.bass as bass  # noqa: F401
    from concourse import bacc
    import concourse.mybir as mybir
    import concourse.tile as tile

    f32 = mybir.dt.float32
    f32r = mybir.dt.float32r
    AF = mybir.ActivationFunctionType
    OP = mybir.AluOpType

    def R(ap):
        return ap.bitcast(f32r)

    nc = bacc.Bacc("TRN2", target_bir_lowering=False)

    h0_d = nc.declare_dram_parameter("h0", [N, D], f32, isOutput=False)
    wq_d = nc.declare_dram_parameter("wq", [L, D, D], f32r, isOutput=False)
    wk_d = nc.declare_dram_parameter("wk", [L, D, D], f32r, isOutput=False)
    wv_d = nc.declare_dram_parameter("wv", [L, D, D], f32r, isOutput=False)
    w1_d = nc.declare_dram_parameter("w1", [L, D, D], f32r, isOutput=False)
    w2_d = nc.declare_dram_parameter("w2", [L, D, D], f32r, isOutput=False)
    tri_d = nc.declare_dram_parameter("tri", [P, P], f32, isOutput=False)
    ident_d = nc.declare_dram_parameter("ident", [P, P], f32, isOutput=False)
    invpos_d = nc.declare_dram_parameter("invpos", [P, NT], f32, isOutput=False)
    if use_b1:
        b1_d = nc.declare_dram_parameter("b1", [L, D], f32, isOutput=False)
    if use_b2:
        b2_d = nc.declare_dram_parameter("b2", [L, D], f32, isOutput=False)
    if use_ln1_gb:
        ln1g_d = nc.declare_dram_parameter("ln1g", [L, D], f32, isOutput=False)
        ln1b_d = nc.declare_dram_parameter("ln1b", [L, D], f32, isOutput=False)
    if use_ln2_gb:
        ln2g_d = nc.declare_dram_parameter("ln2g", [L, D], f32, isOutput=False)
        ln2b_d = nc.declare_dram_parameter("ln2b", [L, D], f32, isOutput=False)
    out_d = nc.declare_dram_parameter("out", [N, D], f32, isOutput=True)

    with tile.TileContext(nc) as tc:
        with (
            tc.tile_pool(name="const", bufs=1) as constp,
            tc.tile_pool(name="work", bufs=1) as workp,
            tc.tile_pool(name="stp", bufs=16) as stp,
            tc.tile_pool(name="sqp", bufs=3) as sqp,
            tc.tile_pool(name="small", bufs=8) as smallp,
            tc.tile_pool(name="pbig", bufs=2, space="PSUM") as pbig,
            tc.tile_pool(name="ps512", bufs=2, space="PSUM") as ps512,
            tc.tile_pool(name="pav", bufs=2, space="PSUM") as pavp,
        ):
            # ---------------- constants & weights ----------------
            tri = constp.tile([P, P], f32, tag="tri")
            ident = constp.tile([P, P], f32, tag="ident")
            invpos = constp.tile([P, NT], f32, tag="invpos")
            eps_t = constp.tile([P, 1], f32, tag="eps")
            nc.sync.dma_start(tri[:], tri_d[:])
            nc.sync.dma_start(ident[:], ident_d[:])
            nc.sync.dma_start(invpos[:], invpos_d[:])
            nc.vector.memset(eps_t[:], LN_EPS)

            W = {}
            for wname, wd in (("wq", wq_d), ("wk", wk_d), ("wv", wv_d),
                              ("w1", w1_d), ("w2", w2_d)):
                for l in range(L):
                    for dt_ in range(DT):
                        t = constp.tile([P, D], f32r, tag=f"{wname}_{l}_{dt_}")
                        nc.sync.dma_start(
                            t[:], wd[l, dt_ * P:(dt_ + 1) * P, :])
                        W[wname, l, dt_] = t

            def load_vec(dram, tag):
                out = []
                for l in range(L):
                    t = constp.tile([P, DT], f32, tag=f"{tag}_{l}")
                    nc.sync.dma_start(
                        t[:], dram[l].rearrange("(dt p) -> p dt", p=P))
                    out.append(t)
                return out

            def load_bcast(dram, tag):
                out = []
                for l in range(L):
                    t = constp.tile([P, D], f32, tag=f"{tag}b_{l}")
                    nc.sync.dma_start(
                        t[:], dram[l].unsqueeze(0).to_broadcast([P, D]))
                    out.append(t)
                return out

            b1_t = load_vec(b1_d, "b1") if use_b1 else None
            b2_t = load_bcast(b2_d, "b2") if use_b2 else None
            ln1g_t = load_bcast(ln1g_d, "ln1g") if use_ln1_gb else None
            ln1b_t = load_bcast(ln1b_d, "ln1b") if use_ln1_gb else None
            ln2g_t = load_bcast(ln2g_d, "ln2g") if use_ln2_gb else None
            ln2b_t = load_bcast(ln2b_d, "ln2b") if use_ln2_gb else None

            # ---------------- activations ----------------
            H = []
            for nt in range(NT):
                t = workp.tile([P, D], f32, tag=f"h_{nt}")
                nc.sync.dma_start(t[:], h0_d[nt * P:(nt + 1) * P, :])
                H.append(t)
            HT = [workp.tile([P, N], f32r, tag=f"ht_{d}", name=f"ht_{d}") for d in range(DT)]
            qT = [workp.tile([P, N], f32r, tag=f"qt_{d}", name=f"qt_{d}") for d in range(DT)]
            kT = [workp.tile([P, N], f32r, tag=f"kt_{d}", name=f"kt_{d}") for d in range(DT)]
            h1T = [workp.tile([P, N], f32r, tag=f"h1t_{d}", name=f"h1t_{d}") for d in range(DT)]
            Vp = [workp.tile([P, 2 * D], f32r, tag=f"vp_{i}", name=f"vp_{i}")
                  for i in range(NT // 2)]

            def Vsl(nt):
                return Vp[nt // 2][:, (nt % 2) * D:(nt % 2 + 1) * D]

            def psum_copy(dst, src, act):
                if act:
                    nc.scalar.activation(dst, src, AF.Copy)
                else:
                    nc.vector.tensor_copy(dst, src)

            def transpose_H_to_HT(flip):
                # 8 transposes per 2-bank psum tile, then one wide copy.
                for dt_ in range(DT):
                    for g in range(2):
                        ps = pbig.tile([P, 1024], f32, tag="pbig", name="ps_t")
                        for k in range(8):
                            nt = g * 8 + k
                            nc.tensor.transpose(
                                ps[:, k * P:(k + 1) * P],
                                H[nt][:, dt_ * P:(dt_ + 1) * P],
                                ident[:],
                            )
                        psum_copy(HT[dt_][:, g * 1024:(g + 1) * 1024], ps[:],
                                  act=((dt_ * 2 + g + flip) % 2 == 0))

            def ln_sumsq(h, dst, act):
                # sum(h^2) along free dim -> dst [P,1]; full pass over h.
                if act:
                    sq = sqp.tile([P, D], f32, tag="sqs", name="sqs")
                    nc.scalar.activation(sq[:], h[:], AF.Square, accum_out=dst)
                else:
                    sq = sqp.tile([P, D], f32, tag="sqv", name="sqv")
                    nc.vector.scalar_tensor_tensor(
                        out=sq[:], in0=h[:], scalar=1.0, in1=h[:],
                        op0=OP.mult, op1=OP.mult, accum_out=dst)

            def ln_group_stats(sum_g, ssq_g, n):
                # rstd = 1/sqrt((sumsq - sum^2/D)/D + eps); nmu = -sum/D*rstd
                sq = smallp.tile([P, 4], f32, tag="lnsq", name="lnsq")
                rstd = smallp.tile([P, 4], f32, tag="lnrstd", name="lnrstd")
                nmu = smallp.tile([P, 4], f32, tag="lnnmu", name="lnnmu")
                nc.vector.tensor_tensor(out=sq[:, :n], in0=sum_g[:, :n],
                                        in1=sum_g[:, :n], op=OP.mult)
                nc.vector.scalar_tensor_tensor(
                    out=sq[:, :n], in0=sq[:, :n], scalar=-1.0 / D,
                    in1=ssq_g[:, :n], op0=OP.mult, op1=OP.add)
                nc.scalar.activation(rstd[:, :n], sq[:, :n], AF.Sqrt,
                                     bias=eps_t[:], scale=1.0 / D)
                nc.vector.reciprocal(rstd[:, :n], rstd[:, :n])
                nc.vector.scalar_tensor_tensor(
                    out=nmu[:, :n], in0=sum_g[:, :n], scalar=-1.0 / D,
                    in1=rstd[:, :n], op0=OP.mult, op1=OP.mult)
                return rstd, nmu

            def ln_apply(h, rstd, nmu, k, act, g_t, b_t):
                # h = h*rstd + (-mu*rstd), then optional *g + b
                if act:
                    nc.scalar.activation(h[:], h[:], AF.Identity,
                                         scale=rstd[:, k:k + 1],
                                         bias=nmu[:, k:k + 1])
                else:
                    nc.vector.tensor_scalar(
                        out=h[:], in0=h[:], scalar1=rstd[:, k:k + 1],
                        scalar2=nmu[:, k:k + 1], op0=OP.mult, op1=OP.add)
                if g_t is not None:
                    nc.vector.tensor_tensor(out=h[:], in0=h[:], in1=g_t[:],
                                            op=OP.mult)
                    nc.vector.tensor_tensor(out=h[:], in0=h[:], in1=b_t[:],
                                            op=OP.add)

            for li in range(L * REPEAT):
                l = li % L
                # ---------- phase A: HT + QKV ----------
                transpose_H_to_HT(flip=0)
                for wi, (name, dst) in enumerate((("wq", qT), ("wk", kT))):
                    for et in range(DT):
                        for cp in range(2):
                            ps = pbig.tile([P, 1024], f32, tag="pbig",
                                           name="ps_qk")
                            for half in range(2):
                                ic = cp * 2 + half
                                for dt_ in range(DT):
                                    nc.tensor.matmul(
                                        ps[:, half * 512:(half + 1) * 512],
                                        W[name, l, dt_][:, et * P:(et + 1) * P],
                                        HT[dt_][:, ic * 512:(ic + 1) * 512],
                                        start=(dt_ == 0), stop=(dt_ == DT - 1),
                                        skip_group_check=True,
                                    )
                            psum_copy(dst[et][:, cp * 1024:(cp + 1) * 1024],
                                      ps[:], act=((wi + et + cp) % 2 == 0))
                for pair in range(NT // 2):
                    ps = pbig.tile([P, 1024], f32, tag="pbig", name="ps_v")
                    for k in range(2):
                        nt = 2 * pair + k
                        for dt_ in range(DT):
                            nc.tensor.matmul(
                                ps[:, k * 512:k * 512 + D],
                                HT[dt_][:, nt * P:(nt + 1) * P],
                                W["wv", l, dt_][:],
                                start=(dt_ == 0), stop=(dt_ == DT - 1),
                                skip_group_check=True,
                            )
                    psum_copy(
                        Vp[pair].rearrange("p (b x) -> p b x", b=2),
                        ps.rearrange("p (b x) -> p b x", b=2)[:, :, :D],
                        act=(pair % 2 == 0))

                # ---------- phase B: attention ----------
                for ic in range(IC):
                    STl = []
                    for jt in range(4 * ic + 4):
                        c0 = P * max(0, jt - 4 * ic)
                        # keep produced width >= 256 so fp32r runs 1 cyc/row
                        c0p = min(c0, 512 - 256)
                        ps = ps512.tile([P, 512], f32, tag="ps512",
                                        name="ps_s")
                        for et in range(DT):
                            nc.tensor.matmul(
                                ps[:, c0p:],
                                kT[et][:, jt * P:(jt + 1) * P],
                                qT[et][:, ic * 512 + c0p:(ic + 1) * 512],
                                start=(et == 0), stop=(et == DT - 1),
                            )
                        st = stp.tile([P, 512], f32r, tag="st", name="st")
                        if jt % 2 == 0:
                            nc.scalar.activation(st[:, c0:], ps[:, c0:],
                                                 AF.Relu)
                        else:
                            nc.vector.tensor_scalar_max(st[:, c0:],
                                                        ps[:, c0:], 0.0)
                        if jt >= 4 * ic:
                            eng = nc.gpsimd if TRI_ON_GPSIMD else nc.vector
                            eng.tensor_tensor(
                                out=st[:, c0:c0 + P], in0=st[:, c0:c0 + P],
                                in1=tri[:], op=OP.mult)
                        STl.append(st)
                    sum_g = smallp.tile([P, 4], f32, tag="sumg", name="sumg")
                    ssq_g = smallp.tile([P, 4], f32, tag="ssqg", name="ssqg")
                    for ib_l in range(4):
                        ib = 4 * ic + ib_l
                        av = pavp.tile([P, D], f32, tag="pav", name="pav")
                        for jt in range(ib + 1):
                            nc.tensor.matmul(
                                av[:],
                                STl[jt][:, ib_l * P:(ib_l + 1) * P],
                                Vsl(jt),
                                start=(jt == 0), stop=(jt == ib),
                            )
                        nc.vector.scalar_tensor_tensor(
                            out=H[ib][:], in0=av[:],
                            scalar=invpos[:, ib:ib + 1], in1=H[ib][:],
                            op0=OP.mult, op1=OP.add,
                            accum_out=sum_g[:, ib_l:ib_l + 1])
                        ln_sumsq(H[ib], ssq_g[:, ib_l:ib_l + 1],
                                 act=(ib_l % 2 == 0))
                    rstd, nmu = ln_group_stats(sum_g, ssq_g, 4)
                    for ib_l in range(4):
                        ib = 4 * ic + ib_l
                        ln_apply(H[ib], rstd, nmu, ib_l, act=(ib_l % 2 == 1),
                                 g_t=ln1g_t[l] if use_ln1_gb else None,
                                 b_t=ln1b_t[l] if use_ln1_gb else None)

                # ---------- phase D: MLP ----------
                transpose_H_to_HT(flip=1)
                for et in range(DT):
                    for cp in range(2):
                        ps = pbig.tile([P, 1024], f32, tag="pbig",
                                       name="ps_h1")
                        for half in range(2):
                            ic = cp * 2 + half
                            for dt_ in range(DT):
                                nc.tensor.matmul(
                                    ps[:, half * 512:(half + 1) * 512],
                                    W["w1", l, dt_][:, et * P:(et + 1) * P],
                                    HT[dt_][:, ic * 512:(ic + 1) * 512],
                                    start=(dt_ == 0), stop=(dt_ == DT - 1),
                                    skip_group_check=True,
                                )
                        if use_b1:
                            nc.scalar.activation(
                                h1T[et][:, cp * 1024:(cp + 1) * 1024], ps[:],
                                AF.Relu, bias=b1_t[l][:, et:et + 1])
                        else:
                            nc.scalar.activation(
                                h1T[et][:, cp * 1024:(cp + 1) * 1024], ps[:],
                                AF.Relu)
                for g in range(NT // 4):
                    sum_g = smallp.tile([P, 4], f32, tag="sumg", name="sumg2")
                    ssq_g = smallp.tile([P, 4], f32, tag="ssqg", name="ssqg2")
                    for k in range(4):
                        nt = 4 * g + k
                        ps = pavp.tile([P, D], f32, tag="pav", name="ps_m")
                        for et in range(DT):
                            nc.tensor.matmul(
                                ps[:],
                                h1T[et][:, nt * P:(nt + 1) * P],
                                W["w2", l, et][:],
                                start=(et == 0), stop=(et == DT - 1),
                            )
                        if use_b2:
                            nc.vector.scalar_tensor_tensor(
                                out=H[nt][:], in0=ps[:], scalar=1.0,
                                in1=H[nt][:], op0=OP.mult, op1=OP.add)
                            nc.vector.tensor_tensor(out=H[nt][:], in0=H[nt][:],
                                                    in1=b2_t[l][:], op=OP.add)
                            nc.vector.tensor_reduce(
                                out=sum_g[:, k:k + 1], in_=H[nt][:],
                                axis=mybir.AxisListType.X, op=OP.add)
                        else:
                            nc.vector.scalar_tensor_tensor(
                                out=H[nt][:], in0=ps[:], scalar=1.0,
                                in1=H[nt][:], op0=OP.mult, op1=OP.add,
                                accum_out=sum_g[:, k:k + 1])
                        ln_sumsq(H[nt], ssq_g[:, k:k + 1], act=(k % 2 == 0))
                    rstd, nmu = ln_group_stats(sum_g, ssq_g, 4)
                    for k in range(4):
                        nt = 4 * g + k
                        ln_apply(H[nt], rstd, nmu, k, act=(k % 2 == 1),
                                 g_t=ln2g_t[l] if use_ln2_gb else None,
                                 b_t=ln2b_t[l] if use_ln2_gb else None)

                # write back after last layer
                if li == L * REPEAT - 1:
                    for nt in range(NT):
                        nc.sync.dma_start(
                            out_d[nt * P:(nt + 1) * P, :], H[nt][:])

    nc.finalize()
    return nc


def kernel(**inputs):
    global LAST_EXEC_NS, LAST_RESULTS
    from concourse import bass_utils

    x = np.asarray(inputs["x"], dtype=np.float32)
    wpe = np.asarray(inputs["wpe"], dtype=np.float32)
    assert x.shape == (B, N, D), x.shape

    use_b1 = bool(np.any(np.asarray(inputs["mlp_b1"]) != 0))
    use_b2 = bool(np.any(np.asarray(inputs["mlp_b2"]) != 0))
    use_ln1 = not (np.all(np.asarray(inputs["ln1_g"]) == 1)
                   and np.all(np.asarray(inputs["ln1_b"]) == 0))
    use_ln2 = not (np.all(np.asarray(inputs["ln2_g"]) == 1)
                   and np.all(np.asarray(inputs["ln2_b"]) == 0))

    key = (use_b1, use_b2, use_ln1, use_ln2)
    if key not in _CACHE:
        _CACHE[key] = _build_program(*key)
    nc = _CACHE[key]

    h0 = x + wpe[None, :, :]  # positional embedding folded in on host

    tri = np.tril(np.ones((P, P), dtype=np.float32)).T  # tri[j,i] = j<=i
    ident = np.eye(P, dtype=np.float32)
    pos = np.arange(N, dtype=np.float32).reshape(NT, P).T  # [P, NT]
    invpos = (1.0 / (pos + 1.0)).astype(np.float32)

    shared = {
        "wq": np.ascontiguousarray(inputs["Wq"], dtype=np.float32),
        "wk": np.ascontiguousarray(inputs["Wk"], dtype=np.float32),
        "wv": np.ascontiguousarray(inputs["Wv"], dtype=np.float32),
        "w1": np.ascontiguousarray(inputs["mlp_W1"], dtype=np.float32),
        "w2": np.ascontiguousarray(inputs["mlp_W2"], dtype=np.float32),
        "tri": tri, "ident": ident, "invpos": invpos,
    }
    if use_b1:
        shared["b1"] = np.asarray(inputs["mlp_b1"], dtype=np.float32)
    if use_b2:
        shared["b2"] = np.asarray(inputs["mlp_b2"], dtype=np.float32)
    if use_ln1:
        shared["ln1g"] = np.asarray(inputs["ln1_g"], dtype=np.float32)
        shared["ln1b"] = np.asarray(inputs["ln1_b"], dtype=np.float32)
    if use_ln2:
        shared["ln2g"] = np.asarray(inputs["ln2_g"], dtype=np.float32)
        shared["ln2b"] = np.asarray(inputs["ln2_b"], dtype=np.float32)

    in_maps = [dict(shared, h0=np.ascontiguousarray(h0[c])) for c in range(B)]
    global _last_in_maps
    _last_in_maps = in_maps

    res = bass_utils.run_bass_kernel_spmd(
        nc, in_maps, core_ids=list(range(B)), trace=PROFILE)
    LAST_EXEC_NS = res.exec_time_ns
    LAST_RESULTS = res
    return np.stack([res.results[c]["out"] for c in range(B)], axis=0)
